# revision 1
# baseline (speedup 1.0000x reference)
"""Custom GRU cell kernel for Trainium2, data-parallel over batch on 8 NeuronCores.

Layout strategy: everything on-device lives in [feature=128 partitions, batch free]
("transposed") layout so the six 128x128 weight matrices are the stationary matmul
operands and no on-device transposes are needed. The host pre-transposes x/h0 and
post-transposes the output history.

Per-step dataflow (per core, B_local=256, all tiles [128, 256] unless noted):
  PE : ps_rz[:,0:256]  = W_r.T x_t ; += U_r.T h     (one PSUM bank, [128,512])
       ps_rz[:,256:512]= W_z.T x_t ; += U_z.T h
       ps_xh = W_h.T x_t            (accumulation group left open)
       ps_mmh= U_h.T h
       ps_xh += I.T @ t1            (identity matmul folds the r*(U_h h) add)
  ACT: ru   = sigmoid(ps_rz)        ([128,512], r and u in one op)
       htil = tanh(ps_xh + b_h)
  DVE: t1 = ps_mmh * r ; g = u * dif ; e = g * a_bc ; h' = h + e
  GPS: dif = htil - h ; a_bc = partition_broadcast(a chunk) once per chunk
State h is bf16 and h' is written straight into the output chunk, DMA'd out in
[U, T_chunk, B_local] layout; matmul inputs (x, h, weights) are bf16, PSUM is f32.
"""

import sys

sys.path.insert(0, "/opt/trn_rl_repo")

import numpy as np
import ml_dtypes

import concourse.bass as bass  # noqa: F401  (import registers rust bindings)
import concourse.mybir as mybir
import concourse.tile as tile
from concourse import bacc
from concourse.bass_utils import run_bass_kernel_spmd

BF16 = mybir.dt.bfloat16
F32 = mybir.dt.float32
AF = mybir.ActivationFunctionType
OP = mybir.AluOpType

B, T, U = 2048, 200, 128
NCORES = 8
BL = B // NCORES  # 256 batch rows per core
TC = 25  # timesteps per chunk
NCHUNK = T // TC

# knobs (flipped during tuning)
USE_GPS_BCAST = False  # a-broadcast via gpsimd.partition_broadcast vs PE K=1 matmul
DIF_ON_GPS = False  # (htil - h) on GPSIMD vs VectorE

# set by test.py to collect profile info; kernel() stores results here
PROFILE = False
LAST_RESULT = None
LAST_IN_MAPS = None

_cache = {}


def _build(has_brz: bool, T_=T, TC_=TC, BL_=BL, reps=1):
    """Build + compile the per-core Bass program. has_brz: b_r/b_z nonzero path."""
    NCHUNK_ = T_ // TC_
    nc = bacc.Bacc("TRN2", target_bir_lowering=False)

    xt = nc.dram_tensor("xt", [U, T_, BL_], BF16, kind="ExternalInput")
    av = nc.dram_tensor("av", [T_ * BL_], BF16, kind="ExternalInput")
    h0t = nc.dram_tensor("h0t", [U, BL_], BF16, kind="ExternalInput")
    wcat = nc.dram_tensor("wcat", [6, U, U], BF16, kind="ExternalInput")
    ident_d = nc.dram_tensor("ident", [U, U], BF16, kind="ExternalInput")
    biases = nc.dram_tensor("biases", [U, 3], F32, kind="ExternalInput")
    ones_d = nc.dram_tensor("ones1", [1, U], BF16, kind="ExternalInput")
    outt = nc.dram_tensor("outt", [U, T_, BL_], BF16, kind="ExternalOutput")

    with tile.TileContext(nc) as tc:
        with (
            tc.tile_pool(name="const", bufs=1) as cpool,
            tc.tile_pool(name="xchunk", bufs=2) as xpool,
            tc.tile_pool(name="achunk", bufs=2) as apool,
            tc.tile_pool(name="abc", bufs=2) as abcpool,
            tc.tile_pool(name="ochunk", bufs=2) as opool,
            tc.tile_pool(name="work", bufs=4) as wpool,
            tc.tile_pool(name="psum", bufs=2, space="PSUM") as ppool,
        ):
            wts = []
            for i in range(6):
                wt = cpool.tile([U, U], BF16, tag=f"w{i}")
                nc.sync.dma_start(wt[:], wcat[i])
                wts.append(wt)
            w_r, u_r, w_z, u_z, w_h, u_h = wts
            ident = cpool.tile([U, U], BF16, tag="ident")
            nc.sync.dma_start(ident[:], ident_d[:])
            ones1 = cpool.tile([1, U], BF16, tag="ones1")
            nc.sync.dma_start(ones1[:], ones_d[:])
            btile = cpool.tile([U, 3], F32, tag="biases")
            nc.sync.dma_start(btile[:], biases[:])
            b_r_ap = btile[:, 0:1]
            b_z_ap = btile[:, 1:2]
            b_h_ap = btile[:, 2:3]
            h0tile = cpool.tile([U, BL_], BF16, tag="h0")
            nc.sync.dma_start(h0tile[:], h0t[:])

            for _rep in range(reps):
                xchs = {}

                def load_chunk(k):
                    if k >= NCHUNK_ or k in xchs:
                        return
                    t0, t1x = k * TC_, (k + 1) * TC_
                    xch = xpool.tile([U, TC_, BL_], BF16, tag="xch")
                    nc.sync.dma_start(xch[:], xt[:, t0:t1x, :])
                    ach = apool.tile([1, TC_ * BL_], BF16, tag="ach")
                    nc.sync.dma_start(ach[:], av[t0 * BL_ : t1x * BL_])
                    xchs[k] = (xch, ach)

                def emit_xside(t):
                    """x-dependent matmuls for step t (off the h critical path)."""
                    k, dt = divmod(t, TC_)
                    xch, ach = xchs[k]
                    xs = xch[:, dt, :]
                    ps_rz = ppool.tile([U, 2 * BL_], F32, tag="ps_rz")
                    nc.tensor.matmul(ps_rz[:, 0:BL_], w_r[:], xs, start=True, stop=False)
                    nc.tensor.matmul(ps_rz[:, BL_:], w_z[:], xs, start=False, stop=False)
                    ps_xh_full = ppool.tile([U, 2 * BL_], F32, tag="ps_xh")
                    ps_xh = ps_xh_full[:, 0:BL_]
                    nc.tensor.matmul(ps_xh, w_h[:], xs, start=True, stop=False)
                    ps_a = ppool.tile([U, BL_], F32, tag="ps_a")
                    nc.tensor.matmul(
                        ps_a[:], ones1[:], ach[:, dt * BL_ : (dt + 1) * BL_],
                        start=True, stop=True,
                    )
                    return ps_rz, ps_xh, ps_a

                h_prev = h0tile[:]
                load_chunk(0)
                pending = emit_xside(0)
                och = None
                for t in range(T_):
                    k, dt = divmod(t, TC_)
                    if dt == 0:
                        load_chunk(k + 1)
                        och = opool.tile([U, TC_, BL_], BF16, tag="och")
                    ps_rz, ps_xh, ps_a = pending

                    ps_mmh = ppool.tile([U, BL_], F32, tag="ps_mmh")
                    nc.tensor.matmul(ps_mmh[:], u_h[:], h_prev, start=True, stop=True)
                    nc.tensor.matmul(ps_rz[:, BL_:], u_z[:], h_prev, start=False, stop=False)
                    nc.tensor.matmul(ps_rz[:, 0:BL_], u_r[:], h_prev, start=False, stop=True)

                    r_sb = wpool.tile([U, BL_], BF16, tag="r_sb")
                    if has_brz:
                        nc.scalar.activation(r_sb[:], ps_rz[:, 0:BL_], AF.Sigmoid, bias=b_r_ap)
                    else:
                        nc.scalar.activation(r_sb[:], ps_rz[:, 0:BL_], AF.Sigmoid)
                    u_sb = wpool.tile([U, BL_], BF16, tag="u_sb")
                    if has_brz:
                        nc.scalar.activation(u_sb[:], ps_rz[:, BL_:], AF.Sigmoid, bias=b_z_ap)
                    else:
                        nc.scalar.activation(u_sb[:], ps_rz[:, BL_:], AF.Sigmoid)

                    t1 = wpool.tile([U, BL_], BF16, tag="t1")
                    nc.vector.tensor_tensor(t1[:], ps_mmh[:], r_sb[:], OP.mult)
                    nc.tensor.matmul(ps_xh, ident[:], t1[:], start=False, stop=True)
                    if t + 1 < T_:
                        pending = emit_xside(t + 1)

                    # off-chain while idmm/tanh run: uhat = a*u, m1 = (uhat-1)*h
                    uhat = wpool.tile([U, BL_], BF16, tag="uhat")
                    nc.vector.tensor_tensor(uhat[:], u_sb[:], ps_a[:], OP.mult)
                    m1 = wpool.tile([U, BL_], BF16, tag="m1")
                    nc.vector.scalar_tensor_tensor(
                        m1[:], uhat[:], 1.0, h_prev, OP.subtract, OP.mult
                    )
                    htil = wpool.tile([U, BL_], BF16, tag="htil")
                    nc.scalar.activation(htil[:], ps_xh, AF.Tanh, bias=b_h_ap)
                    # on-chain tail: hn = uhat*htil - (uhat-1)*h
                    m2 = wpool.tile([U, BL_], BF16, tag="m2")
                    nc.vector.tensor_tensor(m2[:], uhat[:], htil[:], OP.mult)
                    hn = och[:, dt, :]
                    nc.vector.tensor_tensor(hn, m2[:], m1[:], OP.subtract)
                    h_prev = hn

                    if dt == TC_ - 1:
                        nc.sync.dma_start(outt[:, k * TC_ : (k + 1) * TC_, :], och[:])
                        xchs.pop(k, None)

    nc.compile()
    return nc


def kernel(inputs, h0, W_r, U_r, b_r, W_z, U_z, b_z, W_h, U_h, b_h):
    global LAST_RESULT
    inputs = np.asarray(inputs, dtype=np.float32)
    h0 = np.asarray(h0, dtype=np.float32)
    ws = [np.asarray(w, dtype=np.float32) for w in (W_r, U_r, W_z, U_z, W_h, U_h)]
    bs = [np.asarray(b, dtype=np.float32) for b in (b_r, b_z, b_h)]

    has_brz = bool(np.any(bs[0]) or np.any(bs[1]))
    key = has_brz
    if key not in _cache:
        _cache[key] = _build(has_brz)
    nc = _cache[key]

    bf = ml_dtypes.bfloat16
    wcat = np.stack([w.astype(bf) for w in ws])  # [6, U, U]
    ident = np.eye(U, dtype=bf)
    ones1 = np.ones((1, U), dtype=bf)
    biases = np.stack([bs[0], bs[1], bs[2]], axis=1).astype(np.float32)  # [U, 3]

    x = inputs[:, :, :U]  # [B, T, U]
    a = inputs[:, :, U]  # [B, T]

    in_maps = []
    for c in range(NCORES):
        sl = slice(c * BL, (c + 1) * BL)
        xt_c = np.ascontiguousarray(x[sl].transpose(2, 1, 0)).astype(bf)  # [U,T,BL]
        a_c = np.ascontiguousarray(a[sl].T).astype(bf).reshape(T * BL)  # [T*BL]
        h0t_c = np.ascontiguousarray(h0[sl].T).astype(bf)  # [U, BL]
        in_maps.append(
            {
                "xt": xt_c,
                "av": a_c,
                "h0t": h0t_c,
                "wcat": wcat,
                "ident": ident,
                "biases": biases,
                "ones1": ones1,
            }
        )

    res = run_bass_kernel_spmd(nc, in_maps, list(range(NCORES)), trace=PROFILE)
    global LAST_IN_MAPS
    LAST_IN_MAPS = in_maps
    LAST_RESULT = res

    out = np.empty((B, T, U), dtype=np.float32)
    for c in range(NCORES):
        sl = slice(c * BL, (c + 1) * BL)
        # outt: [U, T, BL] bf16 -> [BL, T, U] f32
        out[sl] = res.results[c]["outt"].astype(np.float32).transpose(2, 1, 0)
    return out



# revision 19
# speedup vs baseline: 237.8516x; 237.8516x over previous
"""Custom GRU cell kernel for Trainium2, data-parallel over batch on 8 NeuronCores.

Layout: everything on-device is [feature=128 partitions, batch free] so the six
128x128 weight matrices are stationary matmul operands. Host pre-transposes x/h0
and post-transposes the output history.

The per-step recurrence is critical-path latency-bound, so the kernel is
organized around shortening the h(t-1) -> h(t) chain:

  chain:  U_r.m2 (PE) -> sigmoid_r (ACT) -> t1 = r*mmh (DVE) -> idmm (PE)
          -> tanh (ACT) -> m2 = uhat*htil (DVE) -> [next step]

Key trick (linearity split): h = m2 + m1' with m1' = (1-uhat)*h_prev, and
U h = U m2 + U m1'.  m1' is ready early (it only needs sigmoid_u), so all
U*m1' matmuls run off the critical path; only the U*m2 matmuls wait for tanh.
hn = m2 + m1' is computed off-chain for the output and the next m1'.

The attention weight a is partition-broadcast once per chunk on GPSIMD
(a_bc in SBUF bf16), making uhat = u*a a fast 2x-mode DVE op and freeing the
PE broadcast matmul + PSUM bank the baseline used.

Per-chunk DMAs: x in [U, TC, BL], out [U, TC, BL], a in [1, TC*BL].
Matmul inputs are bf16; PSUM f32; state tensors (m2, m1', hn) bf16.
"""

import sys

sys.path.insert(0, "/opt/trn_rl_repo")

import numpy as np
import ml_dtypes

import concourse.bass as bass  # noqa: F401  (import registers rust bindings)
import concourse.mybir as mybir
import concourse.tile as tile
from concourse import bacc
from concourse.bass_utils import run_bass_kernel_spmd

BF16 = mybir.dt.bfloat16
F32 = mybir.dt.float32
AF = mybir.ActivationFunctionType
OP = mybir.AluOpType

B, T, U = 2048, 200, 128
NCORES = 8
BL = B // NCORES  # 256 batch rows per core
TC = 25  # timesteps per chunk
NCHUNK = T // TC

# set by test.py to collect profile info; kernel() stores results here
PROFILE = False
LAST_RESULT = None
LAST_IN_MAPS = None

_cache = {}

# debug: instruction-name -> semantic label (filled during _build)
LABELS = {}


def _L(inst, label):
    try:
        LABELS[inst.ins.name] = label
    except Exception:
        try:
            LABELS[inst.name] = label
        except Exception:
            pass
    return inst


def _build(has_brz: bool, T_=T, TC_=TC, BL_=BL, reps=1):
    """Build + compile the per-core Bass program. has_brz: b_r/b_z nonzero path."""
    NCHUNK_ = T_ // TC_
    nc = bacc.Bacc("TRN2", target_bir_lowering=False)

    xt = nc.dram_tensor("xt", [U, T_, BL_], BF16, kind="ExternalInput")
    av = nc.dram_tensor("av", [T_ * BL_], BF16, kind="ExternalInput")
    h0t = nc.dram_tensor("h0t", [U, BL_], BF16, kind="ExternalInput")
    # 9 weights: W_r, U_r, W_z, U_z, W_h, U_h, -U_r, -U_z, -U_h
    wcat = nc.dram_tensor("wcat", [9, U, U], BF16, kind="ExternalInput")
    ident_d = nc.dram_tensor("ident", [U, U], BF16, kind="ExternalInput")
    biases = nc.dram_tensor("biases", [U, 3], F32, kind="ExternalInput")
    outt = nc.dram_tensor("outt", [U, T_, BL_], BF16, kind="ExternalOutput")

    with tile.TileContext(nc) as tc:
        with (
            tc.tile_pool(name="const", bufs=1) as cpool,
            tc.tile_pool(name="xchunk", bufs=2) as xpool,
            tc.tile_pool(name="achunk", bufs=2) as apool,
            tc.tile_pool(name="abc", bufs=2) as abcpool,
            tc.tile_pool(name="ochunk", bufs=2) as opool,
            tc.tile_pool(name="work", bufs=4) as wpool,
            tc.tile_pool(name="state", bufs=3) as spool,
            tc.tile_pool(name="psr", bufs=2, space="PSUM") as prpool,
            tc.tile_pool(name="psz", bufs=2, space="PSUM") as pzpool,
            tc.tile_pool(name="psmm", bufs=2, space="PSUM") as pmpool,
            tc.tile_pool(name="psxh", bufs=2, space="PSUM") as pxpool,
        ):
            wts = []
            for i in range(9):
                wt = cpool.tile([U, U], BF16, tag=f"w{i}")
                nc.sync.dma_start(wt[:], wcat[i])
                wts.append(wt)
            w_r, u_r, w_z, u_z, w_h, u_h, nu_r, nu_z, nu_h = wts
            ident = cpool.tile([U, U], BF16, tag="ident")
            nc.sync.dma_start(ident[:], ident_d[:])
            btile = cpool.tile([U, 3], F32, tag="biases")
            nc.sync.dma_start(btile[:], biases[:])
            b_r_ap = btile[:, 0:1]
            b_z_ap = btile[:, 1:2]
            b_h_ap = btile[:, 2:3]
            h0tile = cpool.tile([U, BL_], BF16, tag="h0")
            nc.sync.dma_start(h0tile[:], h0t[:])

            for _rep in range(reps):
                xchs = {}

                def load_chunk(k):
                    if k >= NCHUNK_ or k in xchs:
                        return
                    t0, t1x = k * TC_, (k + 1) * TC_
                    xch = xpool.tile([U, TC_, BL_], BF16, tag="xch")
                    nc.sync.dma_start(xch[:], xt[:, t0:t1x, :])
                    ach = apool.tile([1, TC_ * BL_], BF16, tag="ach")
                    nc.sync.dma_start(ach[:], av[t0 * BL_ : t1x * BL_])
                    abc = abcpool.tile([U, TC_ * BL_], BF16, tag="abc")
                    nc.gpsimd.partition_broadcast(abc[:], ach[:])
                    xchs[k] = (xch, abc)

                def emit_xside(t):
                    """x-dependent matmuls for step t (off the h critical path).
                    Bank pairing avoids PE-write vs ACT/DVE-read serialization
                    on the chain: bank A = [r | mmh], bank B = [z | xh]."""
                    k, dt = divmod(t, TC_)
                    xch, _abc = xchs[k]
                    xs = xch[:, dt, :]
                    ps_rr = prpool.tile([U, BL_], F32, tag="ps_rr")
                    _L(nc.tensor.matmul(
                        ps_rr[:], w_r[:], xs, start=True, stop=False
                    ), f"x_wr[{t}]")
                    ps_zz = pzpool.tile([U, BL_], F32, tag="ps_zz")
                    _L(nc.tensor.matmul(
                        ps_zz[:], w_z[:], xs, start=True, stop=False
                    ), f"x_wz[{t}]")
                    ps_xhh = pxpool.tile([U, BL_], F32, tag="ps_xhh")
                    _L(nc.tensor.matmul(
                        ps_xhh[:], w_h[:], xs, start=True, stop=False
                    ), f"x_wh[{t}]")
                    ps_mm = pmpool.tile([U, BL_], F32, tag="ps_mm")
                    return ps_rr, ps_zz, ps_mm, ps_xhh

                h_prev = h0tile[:]  # step -1 "m2" when m1' absent
                m2_prev = None
                m1p_prev = None
                load_chunk(0)
                pending = emit_xside(0)
                och = None
                for t in range(T_):
                    k, dt = divmod(t, TC_)
                    if dt == 0:
                        load_chunk(k + 1)
                        och = opool.tile([U, TC_, BL_], BF16, tag="och")
                    _abc = xchs[k][1]
                    a_sl = _abc[:, dt * BL_ : (dt + 1) * BL_]
                    ps_rr, ps_zz, ps_mm, ps_xhh = pending
                    ps_r = ps_rr[:]
                    ps_z = ps_zz[:]
                    ps_mmh = ps_mm[:]
                    ps_xh = ps_xhh[:]

                    # --- h-side matmuls ---
                    # m1 part via negated weights: U*m1' = (-U)*m1, m1 = (uhat-1)h
                    # (off-chain: m1(t-1) is ready well before m2(t-1))
                    if m1p_prev is not None:
                        _L(nc.tensor.matmul(
                            ps_mmh, nu_h[:], m1p_prev, start=True, stop=False
                        ), f"nu_h[{t}]")
                        _L(nc.tensor.matmul(
                            ps_r, nu_r[:], m1p_prev, start=False, stop=False
                        ), f"nu_r[{t}]")
                        _L(nc.tensor.matmul(
                            ps_z, nu_z[:], m1p_prev, start=False, stop=False
                        ), f"nu_z[{t}]")
                    # m2 part (chain): r FIRST (own bank closes at ur_m2, so
                    # sigmoid_r waits only this one), then mmh, then z; the
                    # z+mmh bank closes at uz_m2 (before t1/sig_u need it)
                    m2_in = m2_prev if m2_prev is not None else h_prev
                    _L(nc.tensor.matmul(ps_r, u_r[:], m2_in, start=False, stop=True), f"ur_m2[{t}]")
                    _L(nc.tensor.matmul(
                        ps_mmh, u_h[:], m2_in, start=(m1p_prev is None), stop=True
                    ), f"uh_m2[{t}]")
                    _L(nc.tensor.matmul(ps_z, u_z[:], m2_in, start=False, stop=True), f"uz_m2[{t}]")

                    # --- chain: sigmoid_r -> t1 -> idmm -> tanh ---
                    r_sb = wpool.tile([U, BL_], BF16, tag="r_sb")
                    if has_brz:
                        _L(nc.scalar.activation(r_sb[:], ps_r, AF.Sigmoid, bias=b_r_ap), f"sig_r[{t}]")
                    else:
                        _L(nc.scalar.activation(r_sb[:], ps_r, AF.Sigmoid), f"sig_r[{t}]")
                    # off-chain u gate fills ACT between sigmoid_r and tanh;
                    # emitted before idmm so the bank-B tracker orders R->W
                    u_sb = wpool.tile([U, BL_], BF16, tag="u_sb")
                    if has_brz:
                        _L(nc.scalar.activation(u_sb[:], ps_z, AF.Sigmoid, bias=b_z_ap), f"sig_u[{t}]")
                    else:
                        _L(nc.scalar.activation(u_sb[:], ps_z, AF.Sigmoid), f"sig_u[{t}]")

                    # x-side for t+1 fills the PE queue while ACT/DVE work
                    if t + 1 < T_:
                        pending = emit_xside(t + 1)

                    t1 = wpool.tile([U, BL_], BF16, tag="t1")
                    _L(nc.vector.tensor_tensor(t1[:], ps_mmh, r_sb[:], OP.mult), f"t1[{t}]")
                    _L(nc.tensor.matmul(ps_xh, ident[:], t1[:], start=False, stop=True), f"idmm[{t}]")

                    # off-chain: uhat, m1
                    uhat = wpool.tile([U, BL_], BF16, tag="uhat")
                    _L(nc.vector.tensor_tensor(uhat[:], u_sb[:], a_sl, OP.mult), f"uhat[{t}]")
                    # m1 = (uhat - 1) * h_prev  (= -m1'; matmuls use -U weights)
                    m1p = spool.tile([U, BL_], BF16, tag="m1p")
                    _L(nc.vector.scalar_tensor_tensor(
                        m1p[:], uhat[:], 1.0, h_prev, OP.subtract, OP.mult
                    ), f"m1[{t}]")

                    # chain tail: tanh -> m2
                    htil = wpool.tile([U, BL_], BF16, tag="htil")
                    _L(nc.scalar.activation(htil[:], ps_xh, AF.Tanh, bias=b_h_ap), f"tanh[{t}]")
                    m2 = spool.tile([U, BL_], BF16, tag="m2")
                    _L(nc.vector.tensor_tensor(m2[:], uhat[:], htil[:], OP.mult), f"m2[{t}]")

                    # off-chain: hn = m2 - m1 (output + next step's h_prev)
                    hn = och[:, dt, :]
                    _L(nc.vector.tensor_tensor(hn, m2[:], m1p[:], OP.subtract), f"hn[{t}]")

                    h_prev = hn
                    m2_prev = m2[:]
                    m1p_prev = m1p[:]

                    if dt == TC_ - 1:
                        nc.sync.dma_start(outt[:, k * TC_ : (k + 1) * TC_, :], och[:])
                        xchs.pop(k, None)

    nc.compile()
    return nc


def kernel(inputs, h0, W_r, U_r, b_r, W_z, U_z, b_z, W_h, U_h, b_h):
    global LAST_RESULT
    inputs = np.asarray(inputs, dtype=np.float32)
    h0 = np.asarray(h0, dtype=np.float32)
    ws = [np.asarray(w, dtype=np.float32) for w in (W_r, U_r, W_z, U_z, W_h, U_h)]
    bs = [np.asarray(b, dtype=np.float32) for b in (b_r, b_z, b_h)]

    has_brz = bool(np.any(bs[0]) or np.any(bs[1]))
    key = has_brz
    if key not in _cache:
        _cache[key] = _build(has_brz)
    nc = _cache[key]

    bf = ml_dtypes.bfloat16
    # W_r, U_r, W_z, U_z, W_h, U_h, -U_r, -U_z, -U_h
    wlist = ws + [-ws[1], -ws[3], -ws[5]]
    wcat = np.stack([w.astype(bf) for w in wlist])  # [9, U, U]
    ident = np.eye(U, dtype=bf)
    biases = np.stack([bs[0], bs[1], bs[2]], axis=1).astype(np.float32)  # [U, 3]

    x = inputs[:, :, :U]  # [B, T, U]
    a = inputs[:, :, U]  # [B, T]

    in_maps = []
    for c in range(NCORES):
        sl = slice(c * BL, (c + 1) * BL)
        xt_c = np.ascontiguousarray(x[sl].transpose(2, 1, 0)).astype(bf)  # [U,T,BL]
        a_c = np.ascontiguousarray(a[sl].T).astype(bf).reshape(T * BL)  # [T*BL]
        h0t_c = np.ascontiguousarray(h0[sl].T).astype(bf)  # [U, BL]
        in_maps.append(
            {
                "xt": xt_c,
                "av": a_c,
                "h0t": h0t_c,
                "wcat": wcat,
                "ident": ident,
                "biases": biases,
            }
        )

    res = run_bass_kernel_spmd(nc, in_maps, list(range(NCORES)), trace=PROFILE)
    global LAST_IN_MAPS
    LAST_IN_MAPS = in_maps
    LAST_RESULT = res

    out = np.empty((B, T, U), dtype=np.float32)
    for c in range(NCORES):
        sl = slice(c * BL, (c + 1) * BL)
        # outt: [U, T, BL] bf16 -> [BL, T, U] f32
        out[sl] = res.results[c]["outt"].astype(np.float32).transpose(2, 1, 0)
    return out
